# revision 11
# baseline (speedup 1.0000x reference)
"""Trainium2 Bass kernel for BondValencePredictor (sparse_attention).

Reference computation (per batch sample a of B=64, A=128 atoms, C=512 in-feats):
    keys    = leaky_relu(x @ Wk + bk, 0.1)                  # [B, A, 256]
    queries = leaky_relu(x @ Wq + bq, 0.1)                  # [B, A, 6144]
              .reshape(B, A, 256, 4, 6)
    bdata[a,b,d,e,f] = sum_c keys[a,b,c] * queries[a,f,c,d,e]
    out = where(f > b, -inf, bdata)                         # [B, A, 4, 6, A]

Sharding: data-parallel over batch — 8 NeuronCores x 8 samples each; weights
replicated, no collectives.

Per-core layout strategy (all matmuls in float32r = full-rate fp32):
  - x is fed transposed: xT [512, 1024] (tokens = 8 samples x 128 atoms), so
    both projections produce channel-major outputs directly (channel on the
    PSUM partition dim, tokens on the free dim, N=512 moving operand).
  - Wq columns are host-permuted from c*24+de to de*256+c so each de-group's
    256 hid-channels are contiguous -> the einsum's rhs slices need no
    on-chip transpose: bdata[b, de, f] = sum_c keysT[c, b] * qT_de[c, f].
  - de (the 4x6 bond-type/valence grid) is processed in 6 blocks of 4; each
    einsum matmul covers one (sample, c-chunk) against 4 de's x 128 atoms
    = N=512 moving, accumulating over the two 128-wide c-chunks in PSUM.
  - The strict upper-triangular mask is applied by adding a 0/-inf tile.
  - Projection blocks are emitted one block ahead of einsum blocks so the
    PE never waits on the activation (Prelu) epilogue.
"""

import numpy as np

import concourse.bass as bass
import concourse.mybir as mybir
from concourse.tile import TileContext
from concourse.bass_utils import run_bass_kernel_spmd

F32 = mybir.dt.float32
F32R = mybir.dt.float32r
AF = mybir.ActivationFunctionType

B, A, CIN = 64, 128, 512
HID = 256
DE = 24                  # 4 bond types x 6 valences
QF = HID * DE            # 6144
NCORES = 8
BPC = B // NCORES        # samples per core
NTOK = BPC * A           # tokens per core
LEAK = 0.1
DBLK = 4                 # de's per block
NBLK = DE // DBLK
KCH = CIN // 128         # contraction chunks


def _split_excess_waits(nc, max_waits=1):
    """Walrus codegen allows only one sem wait per instruction; Tile's
    kernel-tail drain aggregates one wait per logical proc. Hoist excess
    waits onto same-engine drains inserted immediately before (engines
    execute their stream in order, so the happens-before is preserved)."""
    for f in nc.m.functions:
        for bb in f.blocks:
            insts = bb.instructions
            i = 0
            while i < len(insts):
                ins = insts[i]
                si = ins.sync_info
                if si is not None and si.on_wait and len(si.on_wait) > max_waits:
                    waits = list(si.on_wait)
                    extra, keep = waits[:-max_waits], waits[-max_waits:]
                    new_insts = []
                    k = 0
                    while extra:
                        chunk, extra = extra[:max_waits], extra[max_waits:]
                        nd = mybir.InstDrain(
                            name=f"{ins.name}-sw{k}", ins=[], outs=[])
                        nd.engine = ins.engine
                        nd.sync_info = mybir.SyncInfo(on_wait=chunk, on_update=[])
                        new_insts.append(nd)
                        k += 1
                    ins.sync_info = mybir.SyncInfo(
                        on_wait=keep, on_update=list(si.on_update or []))
                    insts[i:i] = new_insts
                    i += len(new_insts)
                i += 1


def _r(ap):
    return ap.bitcast(F32R)


def _build(split_waits=True):
    nc = bass.Bass()
    xt_d = nc.dram_tensor("xt", [CIN, NTOK], F32R, kind="ExternalInput")
    wk_d = nc.dram_tensor("wk", [CIN, HID], F32R, kind="ExternalInput")
    bk_d = nc.dram_tensor("bk2", [128, 2], F32, kind="ExternalInput")
    wq_d = nc.dram_tensor("wq", [CIN, QF], F32R, kind="ExternalInput")
    bq_d = nc.dram_tensor("bq48", [128, 48], F32, kind="ExternalInput")
    mask_d = nc.dram_tensor("mask4", [128, DBLK * A], F32, kind="ExternalInput")
    out_d = nc.dram_tensor("out", [BPC, A, DE, A], F32, kind="ExternalOutput")

    with TileContext(nc) as tc:
        with (
            tc.tile_pool(name="const", bufs=1) as cpool,
            tc.tile_pool(name="wqp", bufs=10) as wqpool,
            tc.tile_pool(name="qtp", bufs=2) as qtpool,
            tc.tile_pool(name="obp", bufs=6) as opool,
            tc.tile_pool(name="psp", bufs=4, space="PSUM") as ps_p,
            tc.tile_pool(name="pse", bufs=4, space="PSUM") as ps_e,
        ):
            # ---- PE warm-up: dummy matmuls with no DMA dependency keep the
            # HAM activity window busy while inputs stream in, so the real
            # matmul stream starts at 2.4 GHz instead of 1.2 GHz ----
            scratch = cpool.tile([128, 512], mybir.dt.bfloat16)
            nc.vector.memset(scratch[:], 0.0)
            ps_w = ps_e.tile([128, 512], F32, name="ps_warm", tag="pe")
            for _ in range(12):
                nc.tensor.matmul(ps_w[:], scratch[:, 0:128], scratch[:],
                                 start=True, stop=True)

            # ---- resident inputs (finest-grained tiles so each matmul is
            # gated only on the chunk it actually reads; wk + tt=0 first) ----
            wk = cpool.tile([128, KCH, HID], F32R)
            nc.sync.dma_start(wk[:], wk_d[:, :].rearrange("(k p) m -> p k m", p=128))
            NTT = NTOK // 512
            xt_t = [[None] * NTT for _ in range(KCH)]
            for tt in range(NTT):
                for k in range(KCH):
                    xt_t[k][tt] = cpool.tile([128, 512], F32R, name=f"xt_{k}_{tt}",
                                             tag=f"xt_{k}_{tt}")
                    nc.sync.dma_start(
                        xt_t[k][tt][:],
                        xt_d[k * 128:(k + 1) * 128, tt * 512:(tt + 1) * 512])
            bk2 = cpool.tile([128, 2], F32)
            nc.sync.dma_start(bk2[:], bk_d[:, :])
            bq48 = cpool.tile([128, 48], F32)
            nc.sync.dma_start(bq48[:], bq_d[:, :])

            # ---- keys projection: keysT[c-chunk][c, tok] ----
            # tt-outer so the first half only needs the tt=0 xt chunks
            keysT = cpool.tile([128, 2, NTOK], F32R)
            for tt in range(NTT):
                for hh in range(2):
                    ps = ps_p.tile([128, 512], F32, name=f"psk_{tt}_{hh}",
                                   tag="ps")
                    for k in range(KCH):
                        nc.tensor.matmul(
                            ps[:],
                            wk[:, k, hh * 128:(hh + 1) * 128],
                            xt_t[k][tt][:],
                            start=(k == 0), stop=(k == KCH - 1),
                        )
                    nc.scalar.activation(
                        keysT[:, hh, tt * 512:(tt + 1) * 512], ps[:],
                        AF.Prelu, bias=bk2[:, hh:hh + 1], scale=1.0, alpha=LEAK)
                if tt == 0:
                    # bridge: keep the PE (and HAM) busy while the tt=1 x
                    # chunks and the first wq slices stream in
                    for _ in range(10):
                        nc.tensor.matmul(ps_w[:], scratch[:, 0:128],
                                         scratch[:], start=True, stop=True)

            qt_tiles = {}
            wq_tiles = {}

            def wq_dma(blk, de_i):
                """Issue the weight DMAs for one de (4 x 128KB)."""
                de = blk * DBLK + de_i
                wq_t = []
                for k in range(KCH):
                    w = wqpool.tile([128, HID], F32R,
                                    name=f"wq_{de}_{k}", tag=f"wq_{k}")
                    nc.sync.dma_start(
                        w[:], wq_d[k * 128:(k + 1) * 128,
                                   de * HID:(de + 1) * HID])
                    wq_t.append(w)
                wq_tiles[(blk, de_i)] = wq_t

            def proj_de(blk, de_i):
                """Query projection for one de (16 matmuls, 2 activations)."""
                de = blk * DBLK + de_i
                if de_i == 0:
                    qt_tiles[blk] = qtpool.tile([128, 2, DBLK, NTOK], F32R,
                                                name=f"qt_{blk}", tag="qt")
                qt = qt_tiles[blk]
                wq_t = wq_tiles.pop((blk, de_i))
                for cc in range(2):
                    pss = [ps_p.tile([128, 512], F32,
                                     name=f"psq_{de}_{cc}_{t}", tag="ps")
                           for t in range(NTT)]
                    for k in range(KCH):
                        for tt in range(NTT):
                            nc.tensor.matmul(
                                pss[tt][:],
                                wq_t[k][:, cc * 128:(cc + 1) * 128],
                                xt_t[k][tt][:],
                                start=(k == 0), stop=(k == KCH - 1),
                            )
                    j = de * 2 + cc
                    for tt in range(NTT):
                        nc.scalar.activation(
                            qt[:, cc, de_i, tt * 512:(tt + 1) * 512],
                            pss[tt][:],
                            AF.Prelu, bias=bq48[:, j:j + 1], scale=1.0,
                            alpha=LEAK)

            def einsum_a(blk, a):
                """bdata for one (sample, de-block): 2 matmuls + mask + store."""
                qt = qt_tiles[blk]
                pe = ps_e.tile([128, DBLK * A], F32, name=f"pe_{blk}_{a}",
                               tag="pe")
                for cc in range(2):
                    nc.tensor.matmul(
                        pe[:],
                        keysT[:, cc, a * A:(a + 1) * A],
                        qt[:, cc, :, a * A:(a + 1) * A],
                        start=(cc == 0), stop=(cc == 1),
                    )
                ob = opool.tile([128, DBLK * A], F32, name=f"ob_{blk}_{a}",
                                tag="ob")
                nc.vector.tensor_add(ob[:], pe[:], mask4[:])
                nc.sync.dma_start(
                    out_d[a, :, blk * DBLK:(blk + 1) * DBLK, :],
                    ob[:].rearrange("p (q m) -> p q m", m=A))

            # mask tile is only needed by the first einsum (~40us in);
            # keep it out of the critical early DMA stream
            mask4 = cpool.tile([128, DBLK * A], F32)

            def proj_block(blk):
                for de_i in range(DBLK):
                    wq_dma(blk, de_i)
                if blk == 0:
                    nc.sync.dma_start(mask4[:], mask_d[:, :])
                for de_i in range(DBLK):
                    proj_de(blk, de_i)

            def einsum_block(blk):
                for a in range(BPC):
                    einsum_a(blk, a)

            # one-block software pipeline: proj(blk+1) is emitted before
            # einsum(blk) so the PE never waits on qt's activation epilogue,
            # and each block's weight DMAs are issued before the previous
            # block's out-DMAs can stall the sync engine
            proj_block(0)
            for blk in range(NBLK):
                if blk + 1 < NBLK:
                    proj_block(blk + 1)
                einsum_block(blk)

    if split_waits:
        _split_excess_waits(nc)
    return nc


_NC = None
LAST_RESULTS = None  # BassKernelResults of the most recent kernel() call


def kernel(x, Wk, bk, Wq, bq, _trace=False):
    global _NC, LAST_RESULTS
    if _NC is None:
        _NC = _build()

    x = np.asarray(x, np.float32)
    Wk = np.ascontiguousarray(np.asarray(Wk, np.float32))
    bk = np.asarray(bk, np.float32)
    Wq = np.asarray(Wq, np.float32)
    bq = np.asarray(bq, np.float32)

    # Wq columns c*24+de -> de*256+c; bias into [128, de*2+cc] per-partition form
    wq_perm = np.ascontiguousarray(
        Wq.reshape(CIN, HID, DE).transpose(0, 2, 1).reshape(CIN, QF))
    bq48 = np.ascontiguousarray(
        bq.reshape(2, 128, DE).transpose(1, 2, 0).reshape(128, DE * 2))
    bk2 = np.ascontiguousarray(bk.reshape(2, 128).T)
    m = np.where(np.arange(A)[None, :] > np.arange(A)[:, None],
                 -np.inf, 0.0).astype(np.float32)
    mask4 = np.ascontiguousarray(np.tile(m, (1, DBLK)))

    in_maps = []
    for c in range(NCORES):
        xs = x[c * BPC:(c + 1) * BPC].reshape(NTOK, CIN)
        in_maps.append({
            "xt": np.ascontiguousarray(xs.T),
            "wk": Wk,
            "bk2": bk2,
            "wq": wq_perm,
            "bq48": bq48,
            "mask4": mask4,
        })

    res = run_bass_kernel_spmd(_NC, in_maps, core_ids=list(range(NCORES)),
                               trace=_trace)
    LAST_RESULTS = res
    out = np.concatenate([res.results[c]["out"] for c in range(NCORES)], axis=0)
    return np.ascontiguousarray(
        out.reshape(B, A, 4, 6, A)).astype(np.float32, copy=False)


# revision 12
# speedup vs baseline: 1.1086x; 1.1086x over previous
"""Trainium2 Bass kernel for BondValencePredictor (sparse_attention).

Reference computation (per batch sample a of B=64, A=128 atoms, C=512 in-feats):
    keys    = leaky_relu(x @ Wk + bk, 0.1)                  # [B, A, 256]
    queries = leaky_relu(x @ Wq + bq, 0.1)                  # [B, A, 6144]
              .reshape(B, A, 256, 4, 6)
    bdata[a,b,d,e,f] = sum_c keys[a,b,c] * queries[a,f,c,d,e]
    out = where(f > b, -inf, bdata)                         # [B, A, 4, 6, A]

Sharding: data-parallel over batch — 8 NeuronCores x 8 samples each; weights
replicated, no collectives.

Per-core layout strategy (all matmuls in float32r = full-rate fp32):
  - x is fed transposed: xT [512, 1024] (tokens = 8 samples x 128 atoms), so
    both projections produce channel-major outputs directly (channel on the
    PSUM partition dim, tokens on the free dim, N=512 moving operand).
  - Wq columns are host-permuted from c*24+de to de*256+c so each de-group's
    256 hid-channels are contiguous -> the einsum's rhs slices need no
    on-chip transpose: bdata[b, de, f] = sum_c keysT[c, b] * qT_de[c, f].
  - de (the 4x6 bond-type/valence grid) is processed in 6 blocks of 4; each
    einsum matmul covers one (sample, c-chunk) against 4 de's x 128 atoms
    = N=512 moving, accumulating over the two 128-wide c-chunks in PSUM.
  - The strict upper-triangular mask is applied by adding a 0/-inf tile.
  - Projection blocks are emitted one block ahead of einsum blocks so the
    PE never waits on the activation (Prelu) epilogue.
"""

import numpy as np

import concourse.bass as bass
import concourse.mybir as mybir
from concourse.tile import TileContext
from concourse.bass_utils import run_bass_kernel_spmd

F32 = mybir.dt.float32
F32R = mybir.dt.float32r
AF = mybir.ActivationFunctionType

B, A, CIN = 64, 128, 512
HID = 256
DE = 24                  # 4 bond types x 6 valences
QF = HID * DE            # 6144
NCORES = 8
BPC = B // NCORES        # samples per core
NTOK = BPC * A           # tokens per core
LEAK = 0.1
DBLK = 4                 # de's per block
NBLK = DE // DBLK
KCH = CIN // 128         # contraction chunks


def _split_excess_waits(nc, max_waits=1):
    """Walrus codegen allows only one sem wait per instruction; Tile's
    kernel-tail drain aggregates one wait per logical proc. Hoist excess
    waits onto same-engine drains inserted immediately before (engines
    execute their stream in order, so the happens-before is preserved)."""
    for f in nc.m.functions:
        for bb in f.blocks:
            insts = bb.instructions
            i = 0
            while i < len(insts):
                ins = insts[i]
                si = ins.sync_info
                if si is not None and si.on_wait and len(si.on_wait) > max_waits:
                    waits = list(si.on_wait)
                    extra, keep = waits[:-max_waits], waits[-max_waits:]
                    new_insts = []
                    k = 0
                    while extra:
                        chunk, extra = extra[:max_waits], extra[max_waits:]
                        nd = mybir.InstDrain(
                            name=f"{ins.name}-sw{k}", ins=[], outs=[])
                        nd.engine = ins.engine
                        nd.sync_info = mybir.SyncInfo(on_wait=chunk, on_update=[])
                        new_insts.append(nd)
                        k += 1
                    ins.sync_info = mybir.SyncInfo(
                        on_wait=keep, on_update=list(si.on_update or []))
                    insts[i:i] = new_insts
                    i += len(new_insts)
                i += 1


def _r(ap):
    return ap.bitcast(F32R)


def _build(split_waits=True):
    nc = bass.Bass()
    xt_d = nc.dram_tensor("xt", [CIN, NTOK], F32R, kind="ExternalInput")
    wk_d = nc.dram_tensor("wk", [CIN, HID], F32R, kind="ExternalInput")
    bk_d = nc.dram_tensor("bk2", [128, 2], F32, kind="ExternalInput")
    wq_d = nc.dram_tensor("wq", [CIN, QF], F32R, kind="ExternalInput")
    bq_d = nc.dram_tensor("bq48", [128, 48], F32, kind="ExternalInput")
    mask_d = nc.dram_tensor("mask4", [128, DBLK * A], F32, kind="ExternalInput")
    out_d = nc.dram_tensor("out", [BPC, A, DE, A], F32, kind="ExternalOutput")

    with TileContext(nc) as tc:
        with (
            tc.tile_pool(name="const", bufs=1) as cpool,
            tc.tile_pool(name="wqp", bufs=2) as wqpool,
            tc.tile_pool(name="qtp", bufs=2) as qtpool,
            tc.tile_pool(name="obp", bufs=6) as opool,
            tc.tile_pool(name="psp", bufs=4, space="PSUM") as ps_p,
            tc.tile_pool(name="pse", bufs=4, space="PSUM") as ps_e,
        ):
            # ---- PE warm-up: dummy matmuls with no DMA dependency keep the
            # HAM activity window busy while inputs stream in, so the real
            # matmul stream starts at 2.4 GHz instead of 1.2 GHz ----
            scratch = cpool.tile([128, 512], mybir.dt.bfloat16)
            nc.vector.memset(scratch[:], 0.0)
            ps_w = ps_e.tile([128, 512], F32, name="ps_warm", tag="pe")
            for _ in range(12):
                nc.tensor.matmul(ps_w[:], scratch[:, 0:128], scratch[:],
                                 start=True, stop=True)

            # ---- resident inputs (finest-grained tiles so each matmul is
            # gated only on the chunk it actually reads; wk + tt=0 first) ----
            wk = cpool.tile([128, KCH, HID], F32R)
            nc.sync.dma_start(wk[:], wk_d[:, :].rearrange("(k p) m -> p k m", p=128))
            NTT = NTOK // 512
            xt_t = [[None] * NTT for _ in range(KCH)]
            for tt in range(NTT):
                for k in range(KCH):
                    xt_t[k][tt] = cpool.tile([128, 512], F32R, name=f"xt_{k}_{tt}",
                                             tag=f"xt_{k}_{tt}")
                    nc.sync.dma_start(
                        xt_t[k][tt][:],
                        xt_d[k * 128:(k + 1) * 128, tt * 512:(tt + 1) * 512])
            bk2 = cpool.tile([128, 2], F32)
            nc.sync.dma_start(bk2[:], bk_d[:, :])
            bq48 = cpool.tile([128, 48], F32)
            nc.sync.dma_start(bq48[:], bq_d[:, :])

            # ---- keys projection: keysT[c-chunk][c, tok] ----
            # tt-outer so the first half only needs the tt=0 xt chunks
            keysT = cpool.tile([128, 2, NTOK], F32R)
            for tt in range(NTT):
                for hh in range(2):
                    ps = ps_p.tile([128, 512], F32, name=f"psk_{tt}_{hh}",
                                   tag="ps")
                    for k in range(KCH):
                        nc.tensor.matmul(
                            ps[:],
                            wk[:, k, hh * 128:(hh + 1) * 128],
                            xt_t[k][tt][:],
                            start=(k == 0), stop=(k == KCH - 1),
                        )
                    nc.scalar.activation(
                        keysT[:, hh, tt * 512:(tt + 1) * 512], ps[:],
                        AF.Prelu, bias=bk2[:, hh:hh + 1], scale=1.0, alpha=LEAK)
                if tt == 0:
                    # bridge: keep the PE (and HAM) busy while the tt=1 x
                    # chunks and the first wq slices stream in
                    for _ in range(10):
                        nc.tensor.matmul(ps_w[:], scratch[:, 0:128],
                                         scratch[:], start=True, stop=True)

            qt_tiles = {}
            wq_tiles = {}

            def wq_dma(blk):
                """Issue the weight DMAs for one block (4 x 512KB, 4KB rows —
                per-de slices would halve the DMA descriptor size and tank
                aggregate HBM throughput)."""
                wq_t = []
                for k in range(KCH):
                    w = wqpool.tile([128, DBLK * HID], F32R,
                                    name=f"wq_{blk}_{k}", tag=f"wq_{k}")
                    nc.sync.dma_start(
                        w[:], wq_d[k * 128:(k + 1) * 128,
                                   blk * DBLK * HID:(blk + 1) * DBLK * HID])
                    wq_t.append(w)
                wq_tiles[blk] = wq_t

            def proj_de(blk, de_i):
                """Query projection for one de (16 matmuls, 2 activations)."""
                de = blk * DBLK + de_i
                if de_i == 0:
                    qt_tiles[blk] = qtpool.tile([128, 2, DBLK, NTOK], F32R,
                                                name=f"qt_{blk}", tag="qt")
                qt = qt_tiles[blk]
                wq_t = wq_tiles[blk]
                for cc in range(2):
                    pss = [ps_p.tile([128, 512], F32,
                                     name=f"psq_{de}_{cc}_{t}", tag="ps")
                           for t in range(NTT)]
                    for k in range(KCH):
                        for tt in range(NTT):
                            nc.tensor.matmul(
                                pss[tt][:],
                                wq_t[k][:, de_i * HID + cc * 128:
                                        de_i * HID + (cc + 1) * 128],
                                xt_t[k][tt][:],
                                start=(k == 0), stop=(k == KCH - 1),
                            )
                    j = de * 2 + cc
                    for tt in range(NTT):
                        nc.scalar.activation(
                            qt[:, cc, de_i, tt * 512:(tt + 1) * 512],
                            pss[tt][:],
                            AF.Prelu, bias=bq48[:, j:j + 1], scale=1.0,
                            alpha=LEAK)

            def einsum_a(blk, a):
                """bdata for one (sample, de-block): 2 matmuls + mask + store."""
                qt = qt_tiles[blk]
                pe = ps_e.tile([128, DBLK * A], F32, name=f"pe_{blk}_{a}",
                               tag="pe")
                for cc in range(2):
                    nc.tensor.matmul(
                        pe[:],
                        keysT[:, cc, a * A:(a + 1) * A],
                        qt[:, cc, :, a * A:(a + 1) * A],
                        start=(cc == 0), stop=(cc == 1),
                    )
                ob = opool.tile([128, DBLK * A], F32, name=f"ob_{blk}_{a}",
                                tag="ob")
                nc.vector.tensor_add(ob[:], pe[:], mask4[:])
                nc.sync.dma_start(
                    out_d[a, :, blk * DBLK:(blk + 1) * DBLK, :],
                    ob[:].rearrange("p (q m) -> p q m", m=A))

            # mask tile is only needed by the first einsum (~40us in);
            # keep it out of the critical early DMA stream
            mask4 = cpool.tile([128, DBLK * A], F32)

            def proj_block(blk):
                wq_dma(blk)
                if blk == 0:
                    nc.sync.dma_start(mask4[:], mask_d[:, :])
                for de_i in range(DBLK):
                    proj_de(blk, de_i)

            def einsum_block(blk):
                for a in range(BPC):
                    einsum_a(blk, a)

            # one-block software pipeline: proj(blk+1) is emitted before
            # einsum(blk) so the PE never waits on qt's activation epilogue,
            # and each block's weight DMAs are issued before the previous
            # block's out-DMAs can stall the sync engine
            proj_block(0)
            for blk in range(NBLK):
                if blk + 1 < NBLK:
                    proj_block(blk + 1)
                einsum_block(blk)

    if split_waits:
        _split_excess_waits(nc)
    return nc


_NC = None
LAST_RESULTS = None  # BassKernelResults of the most recent kernel() call


def kernel(x, Wk, bk, Wq, bq, _trace=False):
    global _NC, LAST_RESULTS
    if _NC is None:
        _NC = _build()

    x = np.asarray(x, np.float32)
    Wk = np.ascontiguousarray(np.asarray(Wk, np.float32))
    bk = np.asarray(bk, np.float32)
    Wq = np.asarray(Wq, np.float32)
    bq = np.asarray(bq, np.float32)

    # Wq columns c*24+de -> de*256+c; bias into [128, de*2+cc] per-partition form
    wq_perm = np.ascontiguousarray(
        Wq.reshape(CIN, HID, DE).transpose(0, 2, 1).reshape(CIN, QF))
    bq48 = np.ascontiguousarray(
        bq.reshape(2, 128, DE).transpose(1, 2, 0).reshape(128, DE * 2))
    bk2 = np.ascontiguousarray(bk.reshape(2, 128).T)
    m = np.where(np.arange(A)[None, :] > np.arange(A)[:, None],
                 -np.inf, 0.0).astype(np.float32)
    mask4 = np.ascontiguousarray(np.tile(m, (1, DBLK)))

    in_maps = []
    for c in range(NCORES):
        xs = x[c * BPC:(c + 1) * BPC].reshape(NTOK, CIN)
        in_maps.append({
            "xt": np.ascontiguousarray(xs.T),
            "wk": Wk,
            "bk2": bk2,
            "wq": wq_perm,
            "bq48": bq48,
            "mask4": mask4,
        })

    res = run_bass_kernel_spmd(_NC, in_maps, core_ids=list(range(NCORES)),
                               trace=_trace)
    LAST_RESULTS = res
    out = np.concatenate([res.results[c]["out"] for c in range(NCORES)], axis=0)
    return np.ascontiguousarray(
        out.reshape(B, A, 4, 6, A)).astype(np.float32, copy=False)


# revision 13
# speedup vs baseline: 1.1405x; 1.0288x over previous
"""Trainium2 Bass kernel for BondValencePredictor (sparse_attention).

Reference computation (per batch sample a of B=64, A=128 atoms, C=512 in-feats):
    keys    = leaky_relu(x @ Wk + bk, 0.1)                  # [B, A, 256]
    queries = leaky_relu(x @ Wq + bq, 0.1)                  # [B, A, 6144]
              .reshape(B, A, 256, 4, 6)
    bdata[a,b,d,e,f] = sum_c keys[a,b,c] * queries[a,f,c,d,e]
    out = where(f > b, -inf, bdata)                         # [B, A, 4, 6, A]

Sharding: data-parallel over batch — 8 NeuronCores x 8 samples each; weights
replicated, no collectives.

Per-core layout strategy (all matmuls in float32r = full-rate fp32):
  - x is fed transposed: xT [512, 1024] (tokens = 8 samples x 128 atoms), so
    both projections produce channel-major outputs directly (channel on the
    PSUM partition dim, tokens on the free dim, N=512 moving operand).
  - Wq columns are host-permuted from c*24+de to de*256+c so each de-group's
    256 hid-channels are contiguous -> the einsum's rhs slices need no
    on-chip transpose: bdata[b, de, f] = sum_c keysT[c, b] * qT_de[c, f].
  - de (the 4x6 bond-type/valence grid) is processed in 6 blocks of 4; each
    einsum matmul covers one (sample, c-chunk) against 4 de's x 128 atoms
    = N=512 moving, accumulating over the two 128-wide c-chunks in PSUM.
  - The strict upper-triangular mask is applied by adding a 0/-inf tile.
  - Projection blocks are emitted one block ahead of einsum blocks so the
    PE never waits on the activation (Prelu) epilogue.
"""

import numpy as np

import concourse.bass as bass
import concourse.mybir as mybir
from concourse.tile import TileContext
from concourse.bass_utils import run_bass_kernel_spmd

F32 = mybir.dt.float32
F32R = mybir.dt.float32r
AF = mybir.ActivationFunctionType

B, A, CIN = 64, 128, 512
HID = 256
DE = 24                  # 4 bond types x 6 valences
QF = HID * DE            # 6144
NCORES = 8
BPC = B // NCORES        # samples per core
NTOK = BPC * A           # tokens per core
LEAK = 0.1
DBLK = 4                 # de's per block
NBLK = DE // DBLK
KCH = CIN // 128         # contraction chunks


def _split_excess_waits(nc, max_waits=1):
    """Walrus codegen allows only one sem wait per instruction; Tile's
    kernel-tail drain aggregates one wait per logical proc. Hoist excess
    waits onto same-engine drains inserted immediately before (engines
    execute their stream in order, so the happens-before is preserved)."""
    for f in nc.m.functions:
        for bb in f.blocks:
            insts = bb.instructions
            i = 0
            while i < len(insts):
                ins = insts[i]
                si = ins.sync_info
                if si is not None and si.on_wait and len(si.on_wait) > max_waits:
                    waits = list(si.on_wait)
                    extra, keep = waits[:-max_waits], waits[-max_waits:]
                    new_insts = []
                    k = 0
                    while extra:
                        chunk, extra = extra[:max_waits], extra[max_waits:]
                        nd = mybir.InstDrain(
                            name=f"{ins.name}-sw{k}", ins=[], outs=[])
                        nd.engine = ins.engine
                        nd.sync_info = mybir.SyncInfo(on_wait=chunk, on_update=[])
                        new_insts.append(nd)
                        k += 1
                    ins.sync_info = mybir.SyncInfo(
                        on_wait=keep, on_update=list(si.on_update or []))
                    insts[i:i] = new_insts
                    i += len(new_insts)
                i += 1


def _r(ap):
    return ap.bitcast(F32R)


def _build(split_waits=True):
    nc = bass.Bass()
    xt_d = nc.dram_tensor("xt", [CIN, NTOK], F32R, kind="ExternalInput")
    wk_d = nc.dram_tensor("wk", [CIN, HID], F32R, kind="ExternalInput")
    bk_d = nc.dram_tensor("bk2", [128, 2], F32, kind="ExternalInput")
    wq_d = nc.dram_tensor("wq", [CIN, QF], F32R, kind="ExternalInput")
    bq_d = nc.dram_tensor("bq48", [128, 48], F32, kind="ExternalInput")
    mask_d = nc.dram_tensor("mask4", [128, DBLK * A], F32, kind="ExternalInput")
    out_d = nc.dram_tensor("out", [BPC, A, DE, A], F32, kind="ExternalOutput")

    with TileContext(nc) as tc:
        with (
            tc.tile_pool(name="const", bufs=1) as cpool,
            tc.tile_pool(name="wqp", bufs=2) as wqpool,
            tc.tile_pool(name="qtp", bufs=2) as qtpool,
            tc.tile_pool(name="obp", bufs=6) as opool,
            tc.tile_pool(name="psp", bufs=4, space="PSUM") as ps_p,
            tc.tile_pool(name="pse", bufs=4, space="PSUM") as ps_e,
        ):
            # ---- PE warm-up: dummy matmuls with no DMA dependency keep the
            # HAM activity window busy while inputs stream in, so the real
            # matmul stream starts at 2.4 GHz instead of 1.2 GHz ----
            scratch = cpool.tile([128, 512], mybir.dt.bfloat16)
            nc.vector.memset(scratch[:], 0.0)
            ps_w = ps_e.tile([128, 512], F32, name="ps_warm", tag="pe")
            for _ in range(12):
                nc.tensor.matmul(ps_w[:], scratch[:, 0:128], scratch[:],
                                 start=True, stop=True)

            # ---- resident inputs (finest-grained tiles so each matmul is
            # gated only on the chunk it actually reads; wk + tt=0 first) ----
            wk = cpool.tile([128, KCH, HID], F32R)
            nc.sync.dma_start(wk[:], wk_d[:, :].rearrange("(k p) m -> p k m", p=128))
            NTT = NTOK // 512
            xt_t = [[None] * NTT for _ in range(KCH)]
            for tt in range(NTT):
                for k in range(KCH):
                    xt_t[k][tt] = cpool.tile([128, 512], F32R, name=f"xt_{k}_{tt}",
                                             tag=f"xt_{k}_{tt}")
                    nc.sync.dma_start(
                        xt_t[k][tt][:],
                        xt_d[k * 128:(k + 1) * 128, tt * 512:(tt + 1) * 512])
            bk2 = cpool.tile([128, 2], F32)
            nc.sync.dma_start(bk2[:], bk_d[:, :])
            bq48 = cpool.tile([128, 48], F32)
            nc.sync.dma_start(bq48[:], bq_d[:, :])

            # ---- keys projection: keysT[c-chunk][c, tok] ----
            # tt-outer so the first half only needs the tt=0 xt chunks
            keysT = cpool.tile([128, 2, NTOK], F32R)
            for tt in range(NTT):
                for hh in range(2):
                    ps = ps_p.tile([128, 512], F32, name=f"psk_{tt}_{hh}",
                                   tag="ps")
                    for k in range(KCH):
                        nc.tensor.matmul(
                            ps[:],
                            wk[:, k, hh * 128:(hh + 1) * 128],
                            xt_t[k][tt][:],
                            start=(k == 0), stop=(k == KCH - 1),
                        )
                    nc.scalar.activation(
                        keysT[:, hh, tt * 512:(tt + 1) * 512], ps[:],
                        AF.Prelu, bias=bk2[:, hh:hh + 1], scale=1.0, alpha=LEAK)
                if tt == 0:
                    # bridge: keep the PE (and HAM) busy while the tt=1 x
                    # chunks and the first wq slices stream in
                    for _ in range(10):
                        nc.tensor.matmul(ps_w[:], scratch[:, 0:128],
                                         scratch[:], start=True, stop=True)

            qt_tiles = {}
            wq_tiles = {}

            def wq_dma(blk):
                """Issue the weight DMAs for one block (4 x 512KB, 4KB rows —
                per-de slices would halve the DMA descriptor size and tank
                aggregate HBM throughput)."""
                wq_t = []
                for k in range(KCH):
                    w = wqpool.tile([128, DBLK * HID], F32R,
                                    name=f"wq_{blk}_{k}", tag=f"wq_{k}")
                    nc.sync.dma_start(
                        w[:], wq_d[k * 128:(k + 1) * 128,
                                   blk * DBLK * HID:(blk + 1) * DBLK * HID])
                    wq_t.append(w)
                wq_tiles[blk] = wq_t

            def proj_de(blk, de_i):
                """Query projection for one de (16 matmuls, 2 activations)."""
                de = blk * DBLK + de_i
                if de_i == 0:
                    qt_tiles[blk] = qtpool.tile([128, 2, DBLK, NTOK], F32R,
                                                name=f"qt_{blk}", tag="qt")
                qt = qt_tiles[blk]
                wq_t = wq_tiles[blk]
                for cc in range(2):
                    pss = [ps_p.tile([128, 512], F32,
                                     name=f"psq_{de}_{cc}_{t}", tag="ps")
                           for t in range(NTT)]
                    for k in range(KCH):
                        for tt in range(NTT):
                            nc.tensor.matmul(
                                pss[tt][:],
                                wq_t[k][:, de_i * HID + cc * 128:
                                        de_i * HID + (cc + 1) * 128],
                                xt_t[k][tt][:],
                                start=(k == 0), stop=(k == KCH - 1),
                            )
                    j = de * 2 + cc
                    for tt in range(NTT):
                        nc.scalar.activation(
                            qt[:, cc, de_i, tt * 512:(tt + 1) * 512],
                            pss[tt][:],
                            AF.Prelu, bias=bq48[:, j:j + 1], scale=1.0,
                            alpha=LEAK)

            def einsum_a(blk, a):
                """bdata for one (sample, de-block): 2 matmuls + mask + store."""
                qt = qt_tiles[blk]
                pe = ps_e.tile([128, DBLK * A], F32, name=f"pe_{blk}_{a}",
                               tag="pe")
                for cc in range(2):
                    nc.tensor.matmul(
                        pe[:],
                        keysT[:, cc, a * A:(a + 1) * A],
                        qt[:, cc, :, a * A:(a + 1) * A],
                        start=(cc == 0), stop=(cc == 1),
                    )
                ob = opool.tile([128, DBLK * A], F32, name=f"ob_{blk}_{a}",
                                tag="ob")
                nc.vector.tensor_add(ob[:], pe[:], mask4[:])
                nc.sync.dma_start(
                    out_d[a, :, blk * DBLK:(blk + 1) * DBLK, :],
                    ob[:].rearrange("p (q m) -> p q m", m=A))

            # mask tile is only needed by the first einsum (~40us in);
            # keep it out of the critical early DMA stream
            mask4 = cpool.tile([128, DBLK * A], F32)

            def proj_block(blk):
                wq_dma(blk)
                if blk == 0:
                    nc.sync.dma_start(mask4[:], mask_d[:, :])
                for de_i in range(DBLK):
                    proj_de(blk, de_i)

            def einsum_block(blk):
                for a in range(BPC):
                    einsum_a(blk, a)

            # one-block software pipeline: proj(blk+1) is emitted before
            # einsum(blk) so the PE never waits on qt's activation epilogue,
            # and each block's weight DMAs are issued before the previous
            # block's out-DMAs can stall the sync engine. Einsum samples are
            # spread two-per-de between projection groups so the DVE mask-add
            # epilogue never backpressures the PE.
            proj_block(0)
            for blk in range(NBLK):
                nxt = blk + 1
                if nxt < NBLK:
                    wq_dma(nxt)
                    for de_i in range(DBLK):
                        proj_de(nxt, de_i)
                        einsum_a(blk, 2 * de_i)
                        einsum_a(blk, 2 * de_i + 1)
                else:
                    einsum_block(blk)

    if split_waits:
        _split_excess_waits(nc)
    return nc


_NC = None
LAST_RESULTS = None  # BassKernelResults of the most recent kernel() call


def kernel(x, Wk, bk, Wq, bq, _trace=False):
    global _NC, LAST_RESULTS
    if _NC is None:
        _NC = _build()

    x = np.asarray(x, np.float32)
    Wk = np.ascontiguousarray(np.asarray(Wk, np.float32))
    bk = np.asarray(bk, np.float32)
    Wq = np.asarray(Wq, np.float32)
    bq = np.asarray(bq, np.float32)

    # Wq columns c*24+de -> de*256+c; bias into [128, de*2+cc] per-partition form
    wq_perm = np.ascontiguousarray(
        Wq.reshape(CIN, HID, DE).transpose(0, 2, 1).reshape(CIN, QF))
    bq48 = np.ascontiguousarray(
        bq.reshape(2, 128, DE).transpose(1, 2, 0).reshape(128, DE * 2))
    bk2 = np.ascontiguousarray(bk.reshape(2, 128).T)
    m = np.where(np.arange(A)[None, :] > np.arange(A)[:, None],
                 -np.inf, 0.0).astype(np.float32)
    mask4 = np.ascontiguousarray(np.tile(m, (1, DBLK)))

    in_maps = []
    for c in range(NCORES):
        xs = x[c * BPC:(c + 1) * BPC].reshape(NTOK, CIN)
        in_maps.append({
            "xt": np.ascontiguousarray(xs.T),
            "wk": Wk,
            "bk2": bk2,
            "wq": wq_perm,
            "bq48": bq48,
            "mask4": mask4,
        })

    res = run_bass_kernel_spmd(_NC, in_maps, core_ids=list(range(NCORES)),
                               trace=_trace)
    LAST_RESULTS = res
    out = np.concatenate([res.results[c]["out"] for c in range(NCORES)], axis=0)
    return np.ascontiguousarray(
        out.reshape(B, A, 4, 6, A)).astype(np.float32, copy=False)


# revision 14
# speedup vs baseline: 1.1481x; 1.0067x over previous
"""Trainium2 Bass kernel for BondValencePredictor (sparse_attention).

Reference computation (per batch sample a of B=64, A=128 atoms, C=512 in-feats):
    keys    = leaky_relu(x @ Wk + bk, 0.1)                  # [B, A, 256]
    queries = leaky_relu(x @ Wq + bq, 0.1)                  # [B, A, 6144]
              .reshape(B, A, 256, 4, 6)
    bdata[a,b,d,e,f] = sum_c keys[a,b,c] * queries[a,f,c,d,e]
    out = where(f > b, -inf, bdata)                         # [B, A, 4, 6, A]

Sharding: data-parallel over batch — 8 NeuronCores x 8 samples each; weights
replicated, no collectives.

Per-core layout strategy (all matmuls in float32r = full-rate fp32):
  - x is fed transposed: xT [512, 1024] (tokens = 8 samples x 128 atoms), so
    both projections produce channel-major outputs directly (channel on the
    PSUM partition dim, tokens on the free dim, N=512 moving operand).
  - Wq columns are host-permuted from c*24+de to de*256+c so each de-group's
    256 hid-channels are contiguous -> the einsum's rhs slices need no
    on-chip transpose: bdata[b, de, f] = sum_c keysT[c, b] * qT_de[c, f].
  - de (the 4x6 bond-type/valence grid) is processed in 6 blocks of 4; each
    einsum matmul covers one (sample, c-chunk) against 4 de's x 128 atoms
    = N=512 moving, accumulating over the two 128-wide c-chunks in PSUM.
  - The strict upper-triangular mask is applied by adding a 0/-inf tile.
  - Projection blocks are emitted one block ahead of einsum blocks so the
    PE never waits on the activation (Prelu) epilogue.
"""

import numpy as np

import concourse.bass as bass
import concourse.mybir as mybir
from concourse.tile import TileContext
from concourse.bass_utils import run_bass_kernel_spmd

F32 = mybir.dt.float32
F32R = mybir.dt.float32r
AF = mybir.ActivationFunctionType

B, A, CIN = 64, 128, 512
HID = 256
DE = 24                  # 4 bond types x 6 valences
QF = HID * DE            # 6144
NCORES = 8
BPC = B // NCORES        # samples per core
NTOK = BPC * A           # tokens per core
LEAK = 0.1
DBLK = 4                 # de's per block
NBLK = DE // DBLK
KCH = CIN // 128         # contraction chunks


def _split_excess_waits(nc, max_waits=1):
    """Walrus codegen allows only one sem wait per instruction; Tile's
    kernel-tail drain aggregates one wait per logical proc. Hoist excess
    waits onto same-engine drains inserted immediately before (engines
    execute their stream in order, so the happens-before is preserved)."""
    for f in nc.m.functions:
        for bb in f.blocks:
            insts = bb.instructions
            i = 0
            while i < len(insts):
                ins = insts[i]
                si = ins.sync_info
                if si is not None and si.on_wait and len(si.on_wait) > max_waits:
                    waits = list(si.on_wait)
                    extra, keep = waits[:-max_waits], waits[-max_waits:]
                    new_insts = []
                    k = 0
                    while extra:
                        chunk, extra = extra[:max_waits], extra[max_waits:]
                        nd = mybir.InstDrain(
                            name=f"{ins.name}-sw{k}", ins=[], outs=[])
                        nd.engine = ins.engine
                        nd.sync_info = mybir.SyncInfo(on_wait=chunk, on_update=[])
                        new_insts.append(nd)
                        k += 1
                    ins.sync_info = mybir.SyncInfo(
                        on_wait=keep, on_update=list(si.on_update or []))
                    insts[i:i] = new_insts
                    i += len(new_insts)
                i += 1


def _r(ap):
    return ap.bitcast(F32R)


def _build(split_waits=True):
    nc = bass.Bass()
    xt_d = nc.dram_tensor("xt", [CIN, NTOK], F32R, kind="ExternalInput")
    wk_d = nc.dram_tensor("wk", [CIN, HID], F32R, kind="ExternalInput")
    bk_d = nc.dram_tensor("bk2", [128, 2], F32, kind="ExternalInput")
    wq_d = nc.dram_tensor("wq", [CIN, QF], F32R, kind="ExternalInput")
    bq_d = nc.dram_tensor("bq48", [128, 48], F32, kind="ExternalInput")
    mask_d = nc.dram_tensor("mask4", [128, DBLK * A], F32, kind="ExternalInput")
    out_d = nc.dram_tensor("out", [BPC, A, DE, A], F32, kind="ExternalOutput")

    with TileContext(nc) as tc:
        with (
            tc.tile_pool(name="const", bufs=1) as cpool,
            tc.tile_pool(name="wqp", bufs=2) as wqpool,
            tc.tile_pool(name="qtp", bufs=2) as qtpool,
            tc.tile_pool(name="obp", bufs=6) as opool,
            tc.tile_pool(name="psp", bufs=5, space="PSUM") as ps_p,
            tc.tile_pool(name="pse", bufs=3, space="PSUM") as ps_e,
        ):
            # ---- PE warm-up: dummy matmuls with no DMA dependency keep the
            # HAM activity window busy while inputs stream in, so the real
            # matmul stream starts at 2.4 GHz instead of 1.2 GHz ----
            scratch = cpool.tile([128, 512], mybir.dt.bfloat16)
            nc.vector.memset(scratch[:], 0.0)
            ps_w = ps_e.tile([128, 512], F32, name="ps_warm", tag="pe")
            for _ in range(12):
                nc.tensor.matmul(ps_w[:], scratch[:, 0:128], scratch[:],
                                 start=True, stop=True)

            # ---- resident inputs (finest-grained tiles so each matmul is
            # gated only on the chunk it actually reads; wk + tt=0 first) ----
            wk = cpool.tile([128, KCH, HID], F32R)
            nc.sync.dma_start(wk[:], wk_d[:, :].rearrange("(k p) m -> p k m", p=128))
            bk2 = cpool.tile([128, 2], F32)
            nc.sync.dma_start(bk2[:], bk_d[:, :])
            bq48 = cpool.tile([128, 48], F32)
            nc.sync.dma_start(bq48[:], bq_d[:, :])
            NTT = NTOK // 512
            xt_t = [[None] * NTT for _ in range(KCH)]
            for k in range(KCH):
                xt_t[k][0] = cpool.tile([128, 512], F32R, name=f"xt_{k}_0",
                                        tag=f"xt_{k}_0")
                nc.sync.dma_start(
                    xt_t[k][0][:], xt_d[k * 128:(k + 1) * 128, 0:512])
            # block-0 weights interleaved with the tt=1 x chunks: arrival
            # order matches the consumption order of the keys-tt1 matmuls
            # and the first projection de-group
            wq0_t = []
            for k in range(KCH):
                w = wqpool.tile([128, DBLK * HID], F32R, name=f"wq_0_{k}",
                                tag=f"wq_{k}")
                nc.sync.dma_start(w[:], wq_d[k * 128:(k + 1) * 128,
                                             0:DBLK * HID])
                wq0_t.append(w)
                xt_t[k][1] = cpool.tile([128, 512], F32R, name=f"xt_{k}_1",
                                        tag=f"xt_{k}_1")
                nc.sync.dma_start(
                    xt_t[k][1][:], xt_d[k * 128:(k + 1) * 128, 512:1024])

            # ---- keys projection: keysT[c-chunk][c, tok] ----
            # tt-outer so the first half only needs the tt=0 xt chunks
            keysT = cpool.tile([128, 2, NTOK], F32R)
            for tt in range(NTT):
                for hh in range(2):
                    ps = ps_p.tile([128, 512], F32, name=f"psk_{tt}_{hh}",
                                   tag="ps")
                    for k in range(KCH):
                        nc.tensor.matmul(
                            ps[:],
                            wk[:, k, hh * 128:(hh + 1) * 128],
                            xt_t[k][tt][:],
                            start=(k == 0), stop=(k == KCH - 1),
                        )
                    nc.scalar.activation(
                        keysT[:, hh, tt * 512:(tt + 1) * 512], ps[:],
                        AF.Prelu, bias=bk2[:, hh:hh + 1], scale=1.0, alpha=LEAK)
                # bridge: keep the PE (and HAM) busy while the tt=1 x
                # chunks and the first wq slices stream in
                for _ in range(10 if tt == 0 else 6):
                    nc.tensor.matmul(ps_w[:], scratch[:, 0:128],
                                     scratch[:], start=True, stop=True)

            qt_tiles = {}
            wq_tiles = {0: wq0_t}

            def wq_dma(blk):
                """Issue the weight DMAs for one block (4 x 512KB, 4KB rows —
                per-de slices would halve the DMA descriptor size and tank
                aggregate HBM throughput)."""
                wq_t = []
                for k in range(KCH):
                    w = wqpool.tile([128, DBLK * HID], F32R,
                                    name=f"wq_{blk}_{k}", tag=f"wq_{k}")
                    nc.sync.dma_start(
                        w[:], wq_d[k * 128:(k + 1) * 128,
                                   blk * DBLK * HID:(blk + 1) * DBLK * HID])
                    wq_t.append(w)
                wq_tiles[blk] = wq_t

            def proj_de(blk, de_i):
                """Query projection for one de (16 matmuls, 2 activations)."""
                de = blk * DBLK + de_i
                if de_i == 0:
                    qt_tiles[blk] = qtpool.tile([128, 2, DBLK, NTOK], F32R,
                                                name=f"qt_{blk}", tag="qt")
                qt = qt_tiles[blk]
                wq_t = wq_tiles[blk]
                for cc in range(2):
                    pss = [ps_p.tile([128, 512], F32,
                                     name=f"psq_{de}_{cc}_{t}", tag="ps")
                           for t in range(NTT)]
                    for k in range(KCH):
                        for tt in range(NTT):
                            nc.tensor.matmul(
                                pss[tt][:],
                                wq_t[k][:, de_i * HID + cc * 128:
                                        de_i * HID + (cc + 1) * 128],
                                xt_t[k][tt][:],
                                start=(k == 0), stop=(k == KCH - 1),
                            )
                    j = de * 2 + cc
                    for tt in range(NTT):
                        nc.scalar.activation(
                            qt[:, cc, de_i, tt * 512:(tt + 1) * 512],
                            pss[tt][:],
                            AF.Prelu, bias=bq48[:, j:j + 1], scale=1.0,
                            alpha=LEAK)

            def einsum_a(blk, a):
                """bdata for one (sample, de-block): 2 matmuls + mask + store."""
                qt = qt_tiles[blk]
                pe = ps_e.tile([128, DBLK * A], F32, name=f"pe_{blk}_{a}",
                               tag="pe")
                for cc in range(2):
                    nc.tensor.matmul(
                        pe[:],
                        keysT[:, cc, a * A:(a + 1) * A],
                        qt[:, cc, :, a * A:(a + 1) * A],
                        start=(cc == 0), stop=(cc == 1),
                    )
                ob = opool.tile([128, DBLK * A], F32, name=f"ob_{blk}_{a}",
                                tag="ob")
                nc.vector.tensor_add(ob[:], pe[:], mask4[:])
                nc.sync.dma_start(
                    out_d[a, :, blk * DBLK:(blk + 1) * DBLK, :],
                    ob[:].rearrange("p (q m) -> p q m", m=A))

            # mask tile is only needed by the first einsum (~40us in);
            # keep it out of the critical early DMA stream
            mask4 = cpool.tile([128, DBLK * A], F32)

            def proj_block(blk):
                if blk == 0:
                    nc.sync.dma_start(mask4[:], mask_d[:, :])
                else:
                    wq_dma(blk)
                for de_i in range(DBLK):
                    proj_de(blk, de_i)

            def einsum_block(blk):
                for a in range(BPC):
                    einsum_a(blk, a)

            # one-block software pipeline: proj(blk+1) is emitted before
            # einsum(blk) so the PE never waits on qt's activation epilogue,
            # and each block's weight DMAs are issued before the previous
            # block's out-DMAs can stall the sync engine. Einsum samples are
            # spread two-per-de between projection groups so the DVE mask-add
            # epilogue never backpressures the PE.
            proj_block(0)
            for blk in range(NBLK):
                nxt = blk + 1
                if nxt < NBLK:
                    wq_dma(nxt)
                    for de_i in range(DBLK):
                        proj_de(nxt, de_i)
                        einsum_a(blk, 2 * de_i)
                        einsum_a(blk, 2 * de_i + 1)
                else:
                    einsum_block(blk)

    if split_waits:
        _split_excess_waits(nc)
    return nc


_NC = None
LAST_RESULTS = None  # BassKernelResults of the most recent kernel() call


def kernel(x, Wk, bk, Wq, bq, _trace=False):
    global _NC, LAST_RESULTS
    if _NC is None:
        _NC = _build()

    x = np.asarray(x, np.float32)
    Wk = np.ascontiguousarray(np.asarray(Wk, np.float32))
    bk = np.asarray(bk, np.float32)
    Wq = np.asarray(Wq, np.float32)
    bq = np.asarray(bq, np.float32)

    # Wq columns c*24+de -> de*256+c; bias into [128, de*2+cc] per-partition form
    wq_perm = np.ascontiguousarray(
        Wq.reshape(CIN, HID, DE).transpose(0, 2, 1).reshape(CIN, QF))
    bq48 = np.ascontiguousarray(
        bq.reshape(2, 128, DE).transpose(1, 2, 0).reshape(128, DE * 2))
    bk2 = np.ascontiguousarray(bk.reshape(2, 128).T)
    m = np.where(np.arange(A)[None, :] > np.arange(A)[:, None],
                 -np.inf, 0.0).astype(np.float32)
    mask4 = np.ascontiguousarray(np.tile(m, (1, DBLK)))

    in_maps = []
    for c in range(NCORES):
        xs = x[c * BPC:(c + 1) * BPC].reshape(NTOK, CIN)
        in_maps.append({
            "xt": np.ascontiguousarray(xs.T),
            "wk": Wk,
            "bk2": bk2,
            "wq": wq_perm,
            "bq48": bq48,
            "mask4": mask4,
        })

    res = run_bass_kernel_spmd(_NC, in_maps, core_ids=list(range(NCORES)),
                               trace=_trace)
    LAST_RESULTS = res
    out = np.concatenate([res.results[c]["out"] for c in range(NCORES)], axis=0)
    return np.ascontiguousarray(
        out.reshape(B, A, 4, 6, A)).astype(np.float32, copy=False)


# revision 15
# speedup vs baseline: 1.1941x; 1.0400x over previous
"""Trainium2 Bass kernel for BondValencePredictor (sparse_attention).

Reference computation (per batch sample a of B=64, A=128 atoms, C=512 in-feats):
    keys    = leaky_relu(x @ Wk + bk, 0.1)                  # [B, A, 256]
    queries = leaky_relu(x @ Wq + bq, 0.1)                  # [B, A, 6144]
              .reshape(B, A, 256, 4, 6)
    bdata[a,b,d,e,f] = sum_c keys[a,b,c] * queries[a,f,c,d,e]
    out = where(f > b, -inf, bdata)                         # [B, A, 4, 6, A]

Sharding: data-parallel over batch — 8 NeuronCores x 8 samples each; weights
replicated, no collectives.

Per-core layout strategy (all matmuls in float32r = full-rate fp32):
  - x is fed transposed: xT [512, 1024] (tokens = 8 samples x 128 atoms), so
    both projections produce channel-major outputs directly (channel on the
    PSUM partition dim, tokens on the free dim, N=512 moving operand).
  - Wq columns are host-permuted from c*24+de to de*256+c so each de-group's
    256 hid-channels are contiguous -> the einsum's rhs slices need no
    on-chip transpose: bdata[b, de, f] = sum_c keysT[c, b] * qT_de[c, f].
  - de (the 4x6 bond-type/valence grid) is processed in 6 blocks of 4; each
    einsum matmul covers one (sample, c-chunk) against 4 de's x 128 atoms
    = N=512 moving, accumulating over the two 128-wide c-chunks in PSUM.
  - The strict upper-triangular mask is applied by adding a 0/-inf tile.
  - Projection blocks are emitted one block ahead of einsum blocks so the
    PE never waits on the activation (Prelu) epilogue.
"""

import numpy as np

import concourse.bass as bass
import concourse.mybir as mybir
from concourse.tile import TileContext
from concourse.bass_utils import run_bass_kernel_spmd

F32 = mybir.dt.float32
F32R = mybir.dt.float32r
AF = mybir.ActivationFunctionType

B, A, CIN = 64, 128, 512
HID = 256
DE = 24                  # 4 bond types x 6 valences
QF = HID * DE            # 6144
NCORES = 8
BPC = B // NCORES        # samples per core
NTOK = BPC * A           # tokens per core
LEAK = 0.1
DBLK = 4                 # de's per block
NBLK = DE // DBLK
KCH = CIN // 128         # contraction chunks


def _split_excess_waits(nc, max_waits=1):
    """Walrus codegen allows only one sem wait per instruction; Tile's
    kernel-tail drain aggregates one wait per logical proc. Hoist excess
    waits onto same-engine drains inserted immediately before (engines
    execute their stream in order, so the happens-before is preserved)."""
    for f in nc.m.functions:
        for bb in f.blocks:
            insts = bb.instructions
            i = 0
            while i < len(insts):
                ins = insts[i]
                si = ins.sync_info
                if si is not None and si.on_wait and len(si.on_wait) > max_waits:
                    waits = list(si.on_wait)
                    extra, keep = waits[:-max_waits], waits[-max_waits:]
                    new_insts = []
                    k = 0
                    while extra:
                        chunk, extra = extra[:max_waits], extra[max_waits:]
                        nd = mybir.InstNoOp(
                            name=f"{ins.name}-sw{k}", ins=[], outs=[])
                        nd.engine = ins.engine
                        nd.sync_info = mybir.SyncInfo(on_wait=chunk, on_update=[])
                        new_insts.append(nd)
                        k += 1
                    ins.sync_info = mybir.SyncInfo(
                        on_wait=keep, on_update=list(si.on_update or []))
                    insts[i:i] = new_insts
                    i += len(new_insts)
                i += 1


def _r(ap):
    return ap.bitcast(F32R)


def _build(split_waits=True):
    nc = bass.Bass()
    xt_d = nc.dram_tensor("xt", [CIN, NTOK], F32R, kind="ExternalInput")
    wk_d = nc.dram_tensor("wk", [CIN, HID], F32R, kind="ExternalInput")
    bk_d = nc.dram_tensor("bk2", [128, 2], F32, kind="ExternalInput")
    wq_d = nc.dram_tensor("wq", [CIN, QF], F32R, kind="ExternalInput")
    bq_d = nc.dram_tensor("bq48", [128, 48], F32, kind="ExternalInput")
    mask_d = nc.dram_tensor("mask4", [128, DBLK * A], F32, kind="ExternalInput")
    out_d = nc.dram_tensor("out", [BPC, A, DE, A], F32, kind="ExternalOutput")

    with TileContext(nc) as tc:
        with (
            tc.tile_pool(name="const", bufs=1) as cpool,
            tc.tile_pool(name="wqp", bufs=2) as wqpool,
            tc.tile_pool(name="qtp", bufs=2) as qtpool,
            tc.tile_pool(name="obp", bufs=6) as opool,
            tc.tile_pool(name="psp", bufs=5, space="PSUM") as ps_p,
            tc.tile_pool(name="pse", bufs=3, space="PSUM") as ps_e,
        ):
            # ---- PE warm-up: dummy matmuls with no DMA dependency keep the
            # HAM activity window busy while inputs stream in, so the real
            # matmul stream starts at 2.4 GHz instead of 1.2 GHz ----
            scratch = cpool.tile([128, 512], mybir.dt.bfloat16)
            nc.vector.memset(scratch[:], 0.0)
            ps_w = ps_e.tile([128, 512], F32, name="ps_warm", tag="pe")
            for _ in range(12):
                nc.tensor.matmul(ps_w[:], scratch[:, 0:128], scratch[:],
                                 start=True, stop=True)

            # ---- resident inputs (finest-grained tiles so each matmul is
            # gated only on the chunk it actually reads; wk + tt=0 first) ----
            wk = cpool.tile([128, KCH, HID], F32R)
            nc.sync.dma_start(wk[:], wk_d[:, :].rearrange("(k p) m -> p k m", p=128))
            bk2 = cpool.tile([128, 2], F32)
            nc.sync.dma_start(bk2[:], bk_d[:, :])
            bq48 = cpool.tile([128, 48], F32)
            nc.sync.dma_start(bq48[:], bq_d[:, :])
            NTT = NTOK // 512
            xt_t = [[None] * NTT for _ in range(KCH)]
            for k in range(KCH):
                xt_t[k][0] = cpool.tile([128, 512], F32R, name=f"xt_{k}_0",
                                        tag=f"xt_{k}_0")
                nc.sync.dma_start(
                    xt_t[k][0][:], xt_d[k * 128:(k + 1) * 128, 0:512])
            # block-0 weights interleaved with the tt=1 x chunks: arrival
            # order matches the consumption order of the keys-tt1 matmuls
            # and the first projection de-group
            wq0_t = []
            for k in range(KCH):
                w = wqpool.tile([128, DBLK * HID], F32R, name=f"wq_0_{k}",
                                tag=f"wq_{k}")
                nc.sync.dma_start(w[:], wq_d[k * 128:(k + 1) * 128,
                                             0:DBLK * HID])
                wq0_t.append(w)
                xt_t[k][1] = cpool.tile([128, 512], F32R, name=f"xt_{k}_1",
                                        tag=f"xt_{k}_1")
                nc.sync.dma_start(
                    xt_t[k][1][:], xt_d[k * 128:(k + 1) * 128, 512:1024])

            # ---- keys projection: keysT[c-chunk][c, tok] ----
            # tt-outer so the first half only needs the tt=0 xt chunks
            keysT = cpool.tile([128, 2, NTOK], F32R)
            for tt in range(NTT):
                for hh in range(2):
                    ps = ps_p.tile([128, 512], F32, name=f"psk_{tt}_{hh}",
                                   tag="ps")
                    for k in range(KCH):
                        nc.tensor.matmul(
                            ps[:],
                            wk[:, k, hh * 128:(hh + 1) * 128],
                            xt_t[k][tt][:],
                            start=(k == 0), stop=(k == KCH - 1),
                        )
                    nc.scalar.activation(
                        keysT[:, hh, tt * 512:(tt + 1) * 512], ps[:],
                        AF.Prelu, bias=bk2[:, hh:hh + 1], scale=1.0, alpha=LEAK)
                # bridge: keep the PE (and HAM) busy while the tt=1 x
                # chunks and the first wq slices stream in
                for _ in range(10 if tt == 0 else 6):
                    nc.tensor.matmul(ps_w[:], scratch[:, 0:128],
                                     scratch[:], start=True, stop=True)

            qt_tiles = {}
            wq_tiles = {0: wq0_t}

            def wq_dma(blk):
                """Issue the weight DMAs for one block (4 x 512KB, 4KB rows —
                per-de slices would halve the DMA descriptor size and tank
                aggregate HBM throughput)."""
                wq_t = []
                for k in range(KCH):
                    w = wqpool.tile([128, DBLK * HID], F32R,
                                    name=f"wq_{blk}_{k}", tag=f"wq_{k}")
                    nc.sync.dma_start(
                        w[:], wq_d[k * 128:(k + 1) * 128,
                                   blk * DBLK * HID:(blk + 1) * DBLK * HID])
                    wq_t.append(w)
                wq_tiles[blk] = wq_t

            def proj_de(blk, de_i):
                """Query projection for one de (16 matmuls, 2 activations)."""
                de = blk * DBLK + de_i
                if de_i == 0:
                    qt_tiles[blk] = qtpool.tile([128, 2, DBLK, NTOK], F32R,
                                                name=f"qt_{blk}", tag="qt")
                qt = qt_tiles[blk]
                wq_t = wq_tiles[blk]
                for cc in range(2):
                    pss = [ps_p.tile([128, 512], F32,
                                     name=f"psq_{de}_{cc}_{t}", tag="ps")
                           for t in range(NTT)]
                    for k in range(KCH):
                        for tt in range(NTT):
                            nc.tensor.matmul(
                                pss[tt][:],
                                wq_t[k][:, de_i * HID + cc * 128:
                                        de_i * HID + (cc + 1) * 128],
                                xt_t[k][tt][:],
                                start=(k == 0), stop=(k == KCH - 1),
                            )
                    j = de * 2 + cc
                    for tt in range(NTT):
                        nc.scalar.activation(
                            qt[:, cc, de_i, tt * 512:(tt + 1) * 512],
                            pss[tt][:],
                            AF.Prelu, bias=bq48[:, j:j + 1], scale=1.0,
                            alpha=LEAK)

            def einsum_a(blk, a):
                """bdata for one (sample, de-block): 2 matmuls + mask + store."""
                qt = qt_tiles[blk]
                pe = ps_e.tile([128, DBLK * A], F32, name=f"pe_{blk}_{a}",
                               tag="pe")
                for cc in range(2):
                    nc.tensor.matmul(
                        pe[:],
                        keysT[:, cc, a * A:(a + 1) * A],
                        qt[:, cc, :, a * A:(a + 1) * A],
                        start=(cc == 0), stop=(cc == 1),
                    )
                ob = opool.tile([128, DBLK * A], F32, name=f"ob_{blk}_{a}",
                                tag="ob")
                nc.vector.tensor_add(ob[:], pe[:], mask4[:])
                nc.sync.dma_start(
                    out_d[a, :, blk * DBLK:(blk + 1) * DBLK, :],
                    ob[:].rearrange("p (q m) -> p q m", m=A))

            # mask tile is only needed by the first einsum (~40us in);
            # keep it out of the critical early DMA stream
            mask4 = cpool.tile([128, DBLK * A], F32)

            def proj_block(blk):
                if blk == 0:
                    nc.sync.dma_start(mask4[:], mask_d[:, :])
                else:
                    wq_dma(blk)
                for de_i in range(DBLK):
                    proj_de(blk, de_i)

            def einsum_block(blk):
                for a in range(BPC):
                    einsum_a(blk, a)

            # one-block software pipeline: proj(blk+1) is emitted before
            # einsum(blk) so the PE never waits on qt's activation epilogue,
            # and each block's weight DMAs are issued before the previous
            # block's out-DMAs can stall the sync engine. Einsum samples are
            # spread two-per-de between projection groups so the DVE mask-add
            # epilogue never backpressures the PE.
            proj_block(0)
            for blk in range(NBLK):
                nxt = blk + 1
                if nxt < NBLK:
                    wq_dma(nxt)
                    for de_i in range(DBLK):
                        proj_de(nxt, de_i)
                        einsum_a(blk, 2 * de_i)
                        einsum_a(blk, 2 * de_i + 1)
                else:
                    einsum_block(blk)

    if split_waits:
        _split_excess_waits(nc)
    return nc


_NC = None
LAST_RESULTS = None  # BassKernelResults of the most recent kernel() call


def kernel(x, Wk, bk, Wq, bq, _trace=False):
    global _NC, LAST_RESULTS
    if _NC is None:
        _NC = _build()

    x = np.asarray(x, np.float32)
    Wk = np.ascontiguousarray(np.asarray(Wk, np.float32))
    bk = np.asarray(bk, np.float32)
    Wq = np.asarray(Wq, np.float32)
    bq = np.asarray(bq, np.float32)

    # Wq columns c*24+de -> de*256+c; bias into [128, de*2+cc] per-partition form
    wq_perm = np.ascontiguousarray(
        Wq.reshape(CIN, HID, DE).transpose(0, 2, 1).reshape(CIN, QF))
    bq48 = np.ascontiguousarray(
        bq.reshape(2, 128, DE).transpose(1, 2, 0).reshape(128, DE * 2))
    bk2 = np.ascontiguousarray(bk.reshape(2, 128).T)
    m = np.where(np.arange(A)[None, :] > np.arange(A)[:, None],
                 -np.inf, 0.0).astype(np.float32)
    mask4 = np.ascontiguousarray(np.tile(m, (1, DBLK)))

    in_maps = []
    for c in range(NCORES):
        xs = x[c * BPC:(c + 1) * BPC].reshape(NTOK, CIN)
        in_maps.append({
            "xt": np.ascontiguousarray(xs.T),
            "wk": Wk,
            "bk2": bk2,
            "wq": wq_perm,
            "bq48": bq48,
            "mask4": mask4,
        })

    res = run_bass_kernel_spmd(_NC, in_maps, core_ids=list(range(NCORES)),
                               trace=_trace)
    LAST_RESULTS = res
    out = np.concatenate([res.results[c]["out"] for c in range(NCORES)], axis=0)
    return np.ascontiguousarray(
        out.reshape(B, A, 4, 6, A)).astype(np.float32, copy=False)
